# revision 23
# baseline (speedup 1.0000x reference)
"""Trainium2 Bass kernel for nn_DiscriminativeLoss (segment_reduce).

Strategy (pure data parallel, one image per NeuronCore):
  - Device computes, per image, the three heavy segment reductions over the
    1M pixels (K=17 instance-id segments):
      * pass A: counts[k], sums[k, c]            (chunk matmuls vs one-hot)
      * pass B: sum of per-pixel distances to own-segment mean, per k, and
        sum of per-pixel embedding norms, per k  (chunk matmuls vs one-hot)
    The per-pixel distance needs the segment means, which are computed
    on-device between the passes (tiny K-sized math), with the mean table
    broadcast to all partitions for the gather accumulation.
  - Host does the remaining O(K^2) scalar assembly (variance / pairwise
    distance / regularization terms and the batch mean), exactly mirroring
    the reference formulas.

Layout per core: embeddings (4, 1024, 1024) f32 -> bf16 (4, 128, 8192)
[partition dim 128 = pixel-high bits, free 8192 = pixel-low]; mask -> bf16
(128, 8192). One-hot planes are built per j-block with tensor_scalar
is_equal; segment sums contract 128 pixels per TensorE matmul
(lhsT = 5/2 strided columns, rhs = 17 one-hot columns, PSUM accumulate).
"""

import functools
import numpy as np
import ml_dtypes

# ---------------------------------------------------------------- constants
K = 17
C = 4
P = 128
B = 8
H = W = 1024
F = (H * W) // P  # 8192 free columns per image
DELTA_VAR = 0.5
DELTA_DIST = 1.5
ALPHA, BETA, GAMMA = 1.0, 1.0, 0.1

OUT_ROWS = 7  # [sums c=0..3, counts, distseg, normseg]


def legalize_waits(nc):
    """TPB instruction structs hold a single sync-wait slot; strip waits
    that are transitively covered (see callers for the argument)."""
    # Post-pass: TPB instruction structs hold a single sync-wait slot. The
    # x2 slot-reuse touch gets [ACT-self WAW, PE slot-release] waits; the
    # self wait is transitively covered by the PE wait (PE read x2 only
    # after ACT finished writing it), so drop it.
    for bb in nc.m.functions[0].blocks:
        for i in bb.instructions:
            si = getattr(i, "sync_info", None)
            if si is None or len(si.on_wait) < 2:
                continue
            if "Activation" not in str(i.opcode):
                continue
            names = [w.ant_name for w in si.on_wait]
            if any(n.startswith("Activation") for n in names) and any(
                n.startswith("PE") for n in names
            ):
                si.on_wait = [
                    w for w in si.on_wait if not w.ant_name.startswith("Activation")
                ]
    # The kernel-tail Drain waits on every live proc; all compute/input-DMA
    # lanes are transitively covered by the output DMAs (everything feeds
    # the outputs), so keep only the output-DMA queue waits.
    out_sems = set()
    dmas = [
        i
        for bb in nc.m.functions[0].blocks
        for i in bb.instructions
        if "DMACopy" in str(i.opcode) and i.sync_info is not None
    ]
    for i in dmas[-1:]:  # the output DMA is last in program order
        for u in i.sync_info.on_update:
            out_sems.add(u.ant_name)
    for bb in nc.m.functions[0].blocks:
        for i in bb.instructions:
            if "Drain" in str(i.opcode):
                si = i.sync_info
                if si is None:
                    continue
                if len(si.on_wait) > 1:
                    keep = [w for w in si.on_wait if w.ant_name in out_sems]
                    if keep:
                        si.on_wait = keep
    return nc


# ---------------------------------------------------------------- device prog
def build_program(nF=F, nJB=1024):
    """Builds the Bass program. Returns (nc, names)."""
    from concourse import bass, tile
    import concourse.mybir as mybir

    f32 = mybir.dt.float32
    bf16 = mybir.dt.bfloat16
    Alu = mybir.AluOpType
    Act = mybir.ActivationFunctionType

    NB = nF // nJB
    JB = nJB

    nc = bass.Bass()
    e_d = nc.declare_dram_parameter("e", [C + 1, P, nF], bf16, isOutput=False)
    m_d = nc.declare_dram_parameter("m", [P, nF], bf16, isOutput=False)
    o_d = nc.declare_dram_parameter("o", [C + 1, 2 * K], f32, isOutput=True)

    with tile.TileContext(nc) as tc:
        with (
            tc.tile_pool(name="res", bufs=1) as pr,
            tc.tile_pool(name="oh", bufs=2) as poh,
            tc.tile_pool(name="work", bufs=2) as pw,
            tc.tile_pool(name="small", bufs=1) as ps,
            tc.tile_pool(name="psum", bufs=1, space="PSUM") as pp,
        ):
            psumAB = pp.tile([C + 1, 2 * K], f32, tag="psumAB")
            psumA = psumAB[:, 0:K]
            psumB = psumAB[0:2, K : 2 * K]

            # whole-image resident loads (single DMA each)
            er = pr.tile([P, C + 1, nF], bf16, tag="er")
            nc.sync.dma_start(out=er, in_=e_d.transpose([1, 0, 2]))
            mr = pr.tile([P, nF], bf16, tag="mr")
            nc.sync.dma_start(out=mr, in_=m_d[:, :])
            # early 1-elem DVE touches absorb the DMA waits once
            ertouch = pr.tile([1, 2], bf16, tag="ertouch")
            nc.vector.tensor_tensor(
                ertouch[:, 0:1], er[0:1, 0:1, 0:1], er[0:1, 0:1, 0:1], Alu.mult
            )

            # ---------------- pass A: counts + sums ----------------
            for blk in range(NB):
                j0 = blk * JB
                oh = poh.tile([P, K, JB], bf16, tag="oh")
                nc.vector.memset(oh[0:1, 0:1, 0:1], 0.0)  # absorbs slot waits
                for k in range(K):
                    nc.vector.tensor_scalar(
                        oh[:, k, :], mr[:, j0 : j0 + JB], float(k), None, Alu.is_equal
                    )
                for j in range(JB):
                    jj = j0 + j
                    nc.tensor.matmul(
                        psumA,
                        er[:, :, jj],
                        oh[:, :, j],
                        start=(jj == 0),
                        stop=(jj == nF - 1),
                    )

            # ---------------- mid: means + broadcast table ----------------
            a_s = ps.tile([C + 1, K], f32, tag="a_s")
            nc.vector.tensor_copy(a_s, psumA)
            flat = ps.tile([1, (C + 1) * K], f32, tag="flat")
            nc.sync.dma_start(out=flat, in_=a_s[:, :])
            cnt1 = ps.tile([1, K], f32, tag="cnt1")
            nc.vector.tensor_scalar(cnt1, flat[:, C * K :], 1.0, None, Alu.max)
            inv = ps.tile([1, K], f32, tag="inv")
            nc.vector.reciprocal(inv, cnt1)
            mu = ps.tile([1, C * K], f32, tag="mu")
            for c in range(C):
                nc.vector.tensor_tensor(
                    mu[:, c * K : (c + 1) * K], flat[:, c * K : (c + 1) * K],
                    inv, Alu.mult,
                )
            # zero the k=0 (background) means
            for c in range(C):
                nc.vector.memset(mu[:, c * K : c * K + 1], 0.0)
            # V = -2 * mu ; R = sum_c mu^2
            vt = ps.tile([1, C * K], f32, tag="vt")
            nc.vector.tensor_scalar(vt, mu, -2.0, None, Alu.mult)
            sq = ps.tile([1, C * K], f32, tag="sq")
            nc.vector.tensor_tensor(sq, mu, mu, Alu.mult)
            rt = ps.tile([1, K], f32, tag="rt")
            nc.vector.tensor_tensor(rt, sq[:, 0:K], sq[:, K : 2 * K], Alu.add)
            nc.vector.tensor_tensor(rt, rt, sq[:, 2 * K : 3 * K], Alu.add)
            nc.vector.tensor_tensor(rt, rt, sq[:, 3 * K : 4 * K], Alu.add)
            # broadcast [V (68) | R (17)] to all 128 partitions via a
            # rank-1 matmul: ones(1,128).T @ row(1,85) -> PSUM (128,85)
            vr = ps.tile([1, (C + 1) * K], f32, tag="vr")
            nc.vector.tensor_copy(vr[:, : C * K], vt)
            nc.vector.tensor_copy(vr[:, C * K :], rt)
            ones1 = ps.tile([1, P], f32, tag="ones1")
            nc.vector.memset(ones1, 1.0)
            psumT = pp.tile([P, (C + 1) * K], f32, tag="psumT")
            nc.tensor.matmul(psumT, ones1, vr, start=True, stop=True)
            tbl = ps.tile([P, (C + 1) * K], f32, tag="tbl")
            nc.vector.tensor_copy(tbl, psumT)

            # ---------------- pass B: dist + norm segment sums -------------
            for blk in range(NB):
                j0 = blk * JB
                oh = poh.tile([P, K, JB], bf16, tag="oh")
                nc.vector.memset(oh[0:1, 0:1, 0:1], 0.0)  # absorbs slot waits
                for k in range(K):
                    nc.vector.tensor_scalar(
                        oh[:, k, :], mr[:, j0 : j0 + JB], float(k), None, Alu.is_equal
                    )
                ebv = er[:, :, j0 : j0 + JB]
                # Q = sum_c E_c^2 (DVE)
                q = pw.tile([P, JB], bf16, tag="q")
                nc.vector.memset(q[0:1, 0:1], 0.0)  # absorbs slot waits
                sqc = pw.tile([P, JB], bf16, tag="sqc", name="sqc")
                nc.vector.tensor_tensor(q, ebv[:, 0, :], ebv[:, 0, :], Alu.mult)
                for c in range(1, C):
                    nc.vector.tensor_tensor(sqc, ebv[:, c, :], ebv[:, c, :], Alu.mult)
                    nc.vector.tensor_tensor(q, q, sqc, Alu.add)
                # rq = Q + sum_k OH_k * R_k  ; g_c = sum_k OH_k * V_ck
                rq = pw.tile([P, JB], bf16, tag="rq")
                nc.vector.memset(rq[0:1, 0:1], 0.0)  # absorbs slot waits
                nc.vector.scalar_tensor_tensor(
                    rq, oh[:, 1, :], tbl[:, C * K + 1 : C * K + 2], q,
                    Alu.mult, Alu.add,
                )
                for k in range(2, K):
                    nc.vector.scalar_tensor_tensor(
                        rq, oh[:, k, :], tbl[:, C * K + k : C * K + k + 1], rq,
                        Alu.mult, Alu.add,
                    )
                gc = [pw.tile([P, JB], bf16, tag=f"g{c}", name=f"g{c}") for c in range(C)]
                for c in range(C):
                    nc.vector.tensor_scalar(
                        gc[c], oh[:, 1, :], tbl[:, c * K + 1 : c * K + 2],
                        None, Alu.mult,
                    )
                    for k in range(2, K):
                        nc.vector.scalar_tensor_tensor(
                            gc[c], oh[:, k, :], tbl[:, c * K + k : c * K + k + 1],
                            gc[c], Alu.mult, Alu.add,
                        )
                # d2 = rq + sum_c E_c * g_c ; clamp >= 0
                tmp = pw.tile([P, JB], bf16, tag="tmp")
                for c in range(C):
                    nc.vector.tensor_tensor(tmp, ebv[:, c, :], gc[c], Alu.mult)
                    nc.vector.tensor_tensor(rq, rq, tmp, Alu.add)
                nc.vector.tensor_scalar(rq, rq, 0.0, None, Alu.max)
                x2 = pw.tile([P, 2, JB], bf16, tag="x2")
                # two 1-elem ACT touches absorb slot-release + DVE waits
                nc.scalar.activation(x2[0:1, 0:1, 0:1], tbl[0:1, 0:1], Act.Copy)
                nc.scalar.activation(x2[0:1, 0:1, 1:2], rq[0:1, 0:1], Act.Copy)
                nc.scalar.activation(x2[:, 0, :], rq, Act.Sqrt)
                nc.scalar.activation(x2[:, 1, :], q, Act.Sqrt)
                for j in range(JB):
                    jj = j0 + j
                    nc.tensor.matmul(
                        psumB,
                        x2[:, :, j],
                        oh[:, :, j],
                        start=(jj == 0),
                        stop=(jj == nF - 1),
                    )

            # ---------------- outputs ----------------
            o_s = ps.tile([C + 1, 2 * K], f32, tag="o_s")
            nc.vector.tensor_copy(o_s, psumAB)
            nc.sync.dma_start(out=o_d[:, :], in_=o_s)

    return legalize_waits(nc)


@functools.lru_cache(maxsize=1)
def _get_program():
    return build_program()


# ---------------------------------------------------------------- host side
def _prep_core_inputs(emb_b, msk_b):
    """emb_b (C, H, W) f32, msk_b (H, W) int -> device input dict."""
    e = np.empty((C + 1, P, F), dtype=ml_dtypes.bfloat16)
    e[:C] = emb_b.reshape(C, P, F).astype(ml_dtypes.bfloat16)
    e[C] = np.asarray(1.0, dtype=ml_dtypes.bfloat16)
    m = msk_b.reshape(P, F).astype(np.float32).astype(ml_dtypes.bfloat16)
    return {"e": e, "m": m}


def _assemble(dev):
    """dev (C+1, 2K) float -> (loss, has) for one image, mirroring
    the reference per-image formulas."""
    dev = dev.astype(np.float64)  # (C+1, 2K): [:,:K]=A region, [:,K:]=B
    sums = dev[0:C, 0:K].T  # (K, C)
    counts = dev[C, 0:K].copy()
    distseg = dev[0, K:]
    normseg = dev[1, K:]
    counts[0] = 0.0  # background excluded (reference counts[0] == 0)
    present = counts > 0
    safe = np.maximum(counts, 1.0)
    means = np.where(present[:, None], sums / safe[:, None], 0.0)

    mean_dist = distseg / safe
    n = float(present.sum())
    var_terms = np.where(present, np.maximum(mean_dist - DELTA_VAR, 0.0) ** 2, 0.0)
    var_loss = var_terms.sum() / max(n, 1.0)

    d2 = ((means[:, None, :] - means[None, :, :]) ** 2).sum(-1)
    pdist = np.sqrt(np.maximum(d2, 0.0))
    eye = np.eye(K)
    pair_mask = present[:, None] & present[None, :]
    terms = np.maximum(DELTA_DIST - pdist + eye * 1e6, 0.0) ** 2
    dist_loss = (
        np.where(pair_mask, terms, 0.0).sum() / max(n * (n - 1.0), 1.0)
        if n > 1
        else 0.0
    )

    nvalid = counts[1:].sum()
    reg_loss = normseg[1:].sum() / max(nvalid, 1.0)

    loss = ALPHA * var_loss + BETA * dist_loss + GAMMA * reg_loss
    return loss, nvalid > 0


def _combine(per_image):
    losses = np.array([l for l, _ in per_image])
    has = np.array([h for _, h in per_image], dtype=np.float64)
    denom = has.sum()
    if denom > 0:
        return np.float32((losses * has).sum() / max(denom, 1.0))
    return np.float32(0.0)


def kernel(embeddings: np.ndarray, instance_mask: np.ndarray) -> np.ndarray:
    from concourse.bass_utils import run_bass_kernel_spmd

    nc = _get_program()
    nB = embeddings.shape[0]
    in_maps = [
        _prep_core_inputs(np.asarray(embeddings[b]), np.asarray(instance_mask[b]))
        for b in range(nB)
    ]
    res = run_bass_kernel_spmd(nc, in_maps, list(range(nB)))
    per_image = [_assemble(np.asarray(res.results[b]["o"])) for b in range(nB)]
    return _combine(per_image)


def kernel_traced(embeddings: np.ndarray, instance_mask: np.ndarray):
    """Like kernel() but returns (loss, exec_time_ns, profile_result)."""
    from concourse.bass_utils import run_bass_kernel_spmd

    nc = _get_program()
    nB = embeddings.shape[0]
    in_maps = [
        _prep_core_inputs(np.asarray(embeddings[b]), np.asarray(instance_mask[b]))
        for b in range(nB)
    ]
    res = run_bass_kernel_spmd(nc, in_maps, list(range(nB)), trace=True)
    per_image = [_assemble(np.asarray(res.results[b]["o"])) for b in range(nB)]
    return _combine(per_image), res.exec_time_ns, res
